# revision 44
# baseline (speedup 1.0000x reference)
"""Causal multi-head self-attention (B=2, T=2048, D=1024, H=16) on 8 TRN2
NeuronCores.

Sharding (Megatron-style, hardcoded): core = 4*b + g where b in {0,1} is the
batch and g in {0..3} a group of 4 heads. Each core computes Q/K/V projections
for its head group from x[b], fused causal attention for those 4 heads, and a
partial output projection against its 256-column slice of Wo. The host sums
the 4 partial outputs per batch (the all-reduce after out_proj).

v4 design (all matmul operands bf16, fp32 PSUM accumulation):
 - Scores per (kti, head-pair): a 4-way tiled quad of M=64 matmuls covering
   all 16 PE subarray quadrants (rows split by head, cols by k-half); on
   causal-diagonal tiles both the matmul rhs (q columns) and output are
   sliced to the valid q range.
 - PV per (kti, head-pair): col-tiled M=64 pair (head0 -> PSUM rows 0:64,
   head1 -> 64:128) accumulating over kti; these co-issue as one PE stream.
 - Softmax denominators: 4-way col-tiled quad of M=1 ones-matmuls (one per
   head) accumulating into one PSUM bank at partitions {0,32,64,96}; the
   quad co-issues as a single stream.
 - The PE instruction stream is hand-interleaved: projection (next chunks)
   and output-projection (previous chunks) matmul groups are emitted INSIDE
   the attention kti loop, sized so the scalar engine's exp throughput (the
   attention pace-setter) is always covered by independent PE work:
     qc=0: Q/K projections of chunk 1; boundary: V of chunk 1
     qc=1: Q/K+V of chunk 2, then out-proj of chunk 0 (late slots + boundary)
     qc=2: Q/K of chunk 3 + out-proj of chunk 1; boundary: V of chunk 3
     qc=3: out-proj of chunk 2 (delayed past the normalize-2 window);
           out-proj of chunk 3 trails.
 - Normalize: evict PV+sums to SBUF, ONE strided-AP DMA gathers the four
   denominator rows {0,32,64,96} to [16,128], one reciprocal_approx_fast,
   cast to bf16, ONE DMA scatters to a [1,4,512] partition-0 row tile, then
   per head gpsimd partition_broadcast + DVE multiply (bf16 at). For the
   last chunk the multiply reads the PV accumulator directly from PSUM (no
   eviction). The normalize DMAs ride the scalar HWDGE queue; output writes
   ride sync - so the tiny hops never queue behind 256KB output writes.
 - The causal-triangle mask multiply runs on the DVE, NOT gpsimd: gpsimd
   then only ever executes partition_broadcast, avoiding the ~7us
   standard<->attn ucode library swap per normalize; a dummy broadcast at
   startup pre-loads the attn lib during the input-DMA dead time.
 - Keep-warm junk matmuls at phase boundaries and the tail park the PE
   through the normalize latency chain (an idle PE drops to a ~2x-slower
   DVFS p-state for ~3us after restart).
 - Output written in bf16 (host accumulates partials in fp32), halving the
   output DMA traffic.
 - Input DMAs split in 2-slice pieces and alternated across the two HWDGE
   queues in consumption order so the first projection matmul starts ~11us
   in (6.7us of that is fixed engine-preamble).
"""

import numpy as np
import ml_dtypes

import concourse.bass as bass
import concourse.tile as tile
from concourse import bacc, mybir
from concourse.bass_utils import run_bass_kernel_spmd

B, T, D, H, DH = 2, 2048, 1024, 16, 64
HPC = 4  # heads per core
GC = 256  # projection columns per core (HPC * DH)
N_CORES = 8
F32 = mybir.dt.float32
BF16 = mybir.dt.bfloat16
EXP = mybir.ActivationFunctionType.Exp

_CACHE = {}


def _build():
    nc = bacc.Bacc(
        "TRN2", target_bir_lowering=False, debug=False, num_devices=N_CORES
    )
    # Pre-swizzled bf16 inputs (host does transposes + cast):
    #   xs[p, tc, dt, t] = x[b, tc*512+t, dt*128+p]
    #   wq/wk/wv[p, dt, c] = W[g*256+c, dt*128+p]
    #   wo[p, ct, n] = Wo[n, g*256 + ct*128 + p]
    xs = nc.dram_tensor("xs", [128, 4, 8, 512], BF16, kind="ExternalInput").ap()
    wqs = nc.dram_tensor("wqs", [128, 8, GC], BF16, kind="ExternalInput").ap()
    wks = nc.dram_tensor("wks", [128, 8, GC], BF16, kind="ExternalInput").ap()
    wvs = nc.dram_tensor("wvs", [128, 8, GC], BF16, kind="ExternalInput").ap()
    wos = nc.dram_tensor("wos", [128, 2, D], BF16, kind="ExternalInput").ap()
    out = nc.dram_tensor("out", [T, D], BF16, kind="ExternalOutput").ap()

    with tile.TileContext(nc) as tc:
        with (
            tc.tile_pool(name="persist", bufs=1) as persist,
            tc.tile_pool(name="xtp", bufs=4) as xtp,
            tc.tile_pool(name="ptp", bufs=4) as ptp,
            tc.tile_pool(name="normp", bufs=2) as normp,
            tc.tile_pool(name="outp", bufs=2) as outp,
            # PSUM (8 banks): opv0+opv1+sums = 3, st-rotation (2 banks) x2 = 4
            # (shared by attention st tiles and phase-3 po3 tiles), aux = 1
            tc.tile_pool(name="pvp", bufs=1, space="PSUM") as pvp,
            tc.tile_pool(name="stp", bufs=2, space="PSUM") as stp,
            tc.tile_pool(name="auxp", bufs=1, space="PSUM") as auxp,
        ):
            wq = persist.tile([128, 8, GC], BF16, tag="wq")
            wk = persist.tile([128, 8, GC], BF16, tag="wk")
            wv = persist.tile([128, 8, GC], BF16, tag="wv")
            wo = persist.tile([128, 2, D], BF16, tag="wo")
            onesb = persist.tile([128, 4], BF16, tag="onesb")
            onesr = persist.tile([1, 64], BF16, tag="onesr")
            trimask = persist.tile([128, 2, 128], BF16, tag="trimask")
            # per-chunk projection outputs (separate tiles -> no false deps)
            qts = [
                persist.tile([128, 2, 512], BF16, tag=f"qt{t}", name=f"qt{t}")
                for t in range(4)
            ]
            kts = [
                persist.tile([128, 2, 512], BF16, tag=f"kt{t}", name=f"kt{t}")
                for t in range(4)
            ]
            vps = [
                persist.tile([128, 4, HPC, DH], BF16, tag=f"vp{t}", name=f"vp{t}")
                for t in range(4)
            ]
            # normalized attention output per (chunk, head-pair)
            ats = [
                [persist.tile([128, 512], BF16, tag=f"at{t}_{hp}",
                              name=f"at{t}_{hp}") for hp in range(2)]
                for t in range(4)
            ]
            # dedicated diagonal pt tiles per (offset j, head pair hp);
            # cols [0, 128j) are zeroed once and never rewritten
            ptdiag = [
                [persist.tile([128, 2, 512], BF16, tag=f"ptd{j}_{hp}",
                              name=f"ptd{j}_{hp}")
                 for hp in range(2)]
                for j in range(4)
            ]

            # Input DMAs: wq/wk/wv/wo on the scalar queue, x on sync, quartered
            # and interleaved so the first Q-proj matmul (needs wq[:,0,0:128] +
            # xt0[:,0,:]) starts a couple microseconds in.
            xt_all = []
            for t in range(4):
                xti = xtp.tile([128, 8, 512], BF16, tag="xt", name=f"xt{t}")
                xt_all.append(xti)
            # alternate the startup-critical pieces across BOTH HWDGE queues
            # in consumption order so neither queue gates the first matmuls
            nc.sync.dma_start(
                xt_all[0][:, 0:2, :], xs[:, 0, 0:2, :]
            )
            nc.scalar.dma_start(wq[:, 0:2, :], wqs[:, 0:2, :])
            nc.scalar.dma_start(wq[:, 2:4, :], wqs[:, 2:4, :])
            nc.sync.dma_start(
                xt_all[0][:, 2:4, :], xs[:, 0, 2:4, :]
            )
            nc.sync.dma_start(wq[:, 4:6, :], wqs[:, 4:6, :])
            nc.scalar.dma_start(
                xt_all[0][:, 4:6, :], xs[:, 0, 4:6, :]
            )
            nc.scalar.dma_start(wq[:, 6:8, :], wqs[:, 6:8, :])
            nc.sync.dma_start(
                xt_all[0][:, 6:8, :], xs[:, 0, 6:8, :]
            )
            nc.sync.dma_start(wk[:, 0:4, :], wks[:, 0:4, :])
            nc.scalar.dma_start(wk[:, 4:8, :], wks[:, 4:8, :])
            nc.sync.dma_start(xt_all[1][:], xs[:, 1])
            nc.scalar.dma_start(wv[:], wvs[:])
            nc.sync.dma_start(xt_all[2][:], xs[:, 2])
            nc.sync.dma_start(xt_all[3][:], xs[:, 3])
            nc.scalar.dma_start(wo[:], wos[:])

            nc.vector.memset(onesb[:], 1.0)
            nc.vector.memset(onesr[:], 1.0)
            # trimask[r, hh, y] = 1 if y >= r else 0
            nc.vector.memset(trimask[:], 1.0)
            nc.gpsimd.affine_select(
                out=trimask[:],
                in_=trimask[:],
                compare_op=mybir.AluOpType.is_ge,
                fill=0.0,
                base=0,
                pattern=[[0, 2], [1, 128]],
                channel_multiplier=-1,
            )
            for j in range(1, 4):
                for hp in range(2):
                    nc.vector.memset(ptdiag[j][hp][:, :, 0 : 128 * j], 0.0)

            # dummy broadcast: pulls the gpsimd attn ucode lib (the one
            # holding InstPartitionBroadcast) into iram during the startup
            # DMA dead time, so normalize(0) doesn't eat the ~7us lib load
            dumb = persist.tile([64, 4], BF16, tag="dumb")
            nc.gpsimd.partition_broadcast(dumb[:], onesb[0:1, :])

            # PE warm-up: as soon as the first x slice lands, spin throwaway
            # matmuls so the DVFS ramp is fully up when the projections start
            # (a cold PE streams at ~half rate for its first ~3us)
            jw = pvp.tile([128, 512], F32, tag="sums", name="junkw")
            for i in range(10):
                nc.tensor.matmul(
                    jw[:],
                    xt_all[0][:, 0, 0:128],
                    xt_all[0][:, 1, :],
                    start=(i == 0),
                    stop=(i == 9),
                )

            # ---- phase 1 groups: projections for x chunk tci ----
            def phase1_groups(tci):
                xt = xt_all[tci]

                def qk_group(w_sb, dst, ct):
                    def emit():
                        ps = auxp.tile([128, 512], F32, tag="aux", name="ps_qk")
                        for di in range(8):
                            nc.tensor.matmul(
                                ps[:],
                                w_sb[:, di, ct * 128 : (ct + 1) * 128],
                                xt[:, di, :],
                                start=(di == 0),
                                stop=(di == 7),
                            )
                        nc.vector.tensor_copy(dst[:, ct, :], ps[:])
                    return emit

                def v_group(tt):
                    def emit():
                        ps = auxp.tile([128, 256], F32, tag="aux", name="ps_v")
                        for di in range(8):
                            nc.tensor.matmul(
                                ps[:],
                                xt[:, di, tt * 128 : (tt + 1) * 128],
                                wv[:, di, :],
                                start=(di == 0),
                                stop=(di == 7),
                            )
                        nc.vector.tensor_copy(
                            vps[tci][:, tt, :, :],
                            ps[:].rearrange("p (h d) -> p h d", h=HPC),
                        )
                    return emit

                qk = [
                    qk_group(w_sb, dst, ct)
                    for w_sb, dst in ((wq, qts[tci]), (wk, kts[tci]))
                    for ct in range(2)
                ]
                vg = [v_group(tt) for tt in range(4)]
                return qk, vg

            # ---- phase 3 groups: output projection for q chunk qc ----
            def phase3_groups(qc, dma_eng):
                def o_group(tt):
                    def emit():
                        qti = qc * 4 + tt
                        po3 = stp.tile([128, 2, 512], F32, tag="st", name="po3")
                        # ctt-outer: the two nn matmuls of one ctt share the
                        # same stationary lhs -> back-to-back weight reuse
                        for ctt in range(2):
                            for nn in range(2):
                                nc.tensor.matmul(
                                    po3[:, nn, :],
                                    ats[qc][ctt][:, tt * 128 : (tt + 1) * 128],
                                    wo[:, ctt, nn * 512 : (nn + 1) * 512],
                                    start=(ctt == 0),
                                    stop=(ctt == 1),
                                )
                        ot = outp.tile([128, 2, 512], BF16, tag="ot")
                        nc.vector.tensor_copy(ot[:], po3[:])
                        dma_eng.dma_start(
                            out[qti * 128 : (qti + 1) * 128, :].rearrange(
                                "q (a n) -> q a n", a=2
                            ),
                            ot[:],
                        )
                    return emit

                return [o_group(tt) for tt in range(4)]

            # keep-warm filler: the PE drops to a ~2x-slower DVFS p-state
            # after idling, so park it on throwaway matmuls (into the
            # just-freed sums bank) across the normalize latency chain
            # instead of letting it stall cold
            def junk_fill(n):
                # target an stp-pool tile: its previous readers (the exps)
                # are already done at every boundary, so the filler starts
                # instantly -- a sums-bank target would wait on the ssb
                # eviction, which sits behind the stg copies on the DVE
                jt = stp.tile([128, 2, 512], F32, tag="st", name="junk")
                for i in range(n):
                    nc.tensor.matmul(
                        jt[:, 0, :],
                        wo[:, 0, 0:128],
                        wo[:, 1, 0:512],
                        start=(i == 0),
                        stop=(i == n - 1),
                    )

            # ---- phase 2: attention for q chunk qc, interleaving fills ----
            def phase2(qc, fills, boundary, fill_delay=0):
                q0 = qc * 512
                n_kt = 4 * (qc + 1)
                opv = [
                    pvp.tile([128, 512], F32, tag=f"opv{hp}", name=f"opv{hp}")
                    for hp in range(2)
                ]
                sums = pvp.tile([128, 512], F32, tag="sums", name="sums")

                def scores_exp(kti):
                    diag = kti >= 4 * qc
                    j = kti - 4 * qc if diag else 0
                    off = 128 * j
                    tci, tk = divmod(kti, 4)
                    k0 = tk * 128
                    pts = []
                    for hp in range(2):
                        st = stp.tile([128, 2, 512], F32, tag="st", name="st")
                        for hh in range(2):
                            po = 64 * hh
                            # one 64x128 weight tile (row-half po, full cols)
                            # covers both k-halves of this head's score tile
                            nc.tensor.matmul(
                                st[:, hh, off:],
                                kts[tci][po : po + 64, hp, k0 : k0 + 128],
                                qts[qc][po : po + 64, hp, off:],
                                start=True,
                                stop=True,
                                tile_position=(po, 0),
                            )
                        if diag:
                            pt = ptdiag[j][hp]
                            nc.scalar.activation(
                                pt[:, :, off:], st[:, :, off:], EXP, scale=0.125
                            )
                            # mask on DVE, NOT gpsimd: keeps the Pool engine
                            # exclusively on the attn ucode lib (a
                            # standard<->attn lib swap costs ~7us per load)
                            nc.vector.tensor_mul(
                                pt[:, :, off : off + 128],
                                pt[:, :, off : off + 128],
                                trimask[:],
                            )
                        else:
                            pt = ptp.tile([128, 2, 512], BF16, tag="pt")
                            nc.scalar.activation(pt[:], st[:], EXP, scale=0.125)
                        pts.append(pt)
                    return pts

                def pv_sums(kti, pts, last):
                    first = kti == 0
                    tci, tk = divmod(kti, 4)
                    for hp in range(2):
                        for hh in range(2):
                            nc.tensor.matmul(
                                opv[hp][64 * hh : 64 * hh + 64, :],
                                vps[tci][:, tk, 2 * hp + hh, :],
                                pts[hp][:, hh, :],
                                start=first,
                                stop=last,
                                tile_position=(0, 64 * hh),
                            )
                    for q in range(4):
                        hp, hh = divmod(q, 2)
                        nc.tensor.matmul(
                            sums[32 * q : 32 * q + 1, :],
                            onesb[:, q : q + 1],
                            pts[hp][:, hh, :],
                            start=first,
                            stop=last,
                            tile_position=(0, 32 * q),
                        )

                def want(halfstep):
                    lo = 2 * fill_delay
                    if halfstep <= lo:
                        return 0
                    return (halfstep - lo) * len(fills) // (2 * n_kt - lo)

                emitted = 0
                prev = None
                for kti in range(n_kt):
                    cur = scores_exp(kti)
                    while emitted < want(2 * kti + 1):
                        fills[emitted]()
                        emitted += 1
                    if prev is not None:
                        pv_sums(kti - 1, prev, last=False)
                    prev = cur
                    while emitted < want(2 * kti + 2):
                        fills[emitted]()
                        emitted += 1
                pv_sums(n_kt - 1, prev, last=True)
                while emitted < len(fills):
                    fills[emitted]()
                    emitted += 1

                # evict sums FIRST (it heads the normalize critical chain),
                # then the PV accumulators so the next chunk's PV can start
                # during normalization. Last chunk: no next chunk -> the
                # multiplies read the PV accumulator straight from PSUM.
                last_chunk = qc == 3
                stg = []
                if not last_chunk:
                    for q in range(4):
                        hp, hh = divmod(q, 2)
                        s = normp.tile([64, 512], F32, tag=f"stg{q}",
                                       name=f"stg{q}")
                        nc.vector.tensor_copy(
                            s[:], opv[hp][64 * hh : 64 * hh + 64, :]
                        )
                        stg.append(s)
                ssb = normp.tile([128, 512], F32, tag="ssb")
                nc.vector.tensor_copy(ssb[:], sums[:])

                # boundary fills: PE work covering the normalize latency chain
                for g in boundary:
                    g()

                # ONE strided gather of denominator rows {0,32,64,96} ->
                # srec[4q+i, j] = ssb[32q, 128i+j]
                # (on the scalar HWDGE queue: the sync queue carries the
                # bulk output writes, which would delay these tiny hops)
                srec = normp.tile([16, 128], F32, tag="srec")
                nc.scalar.dma_start(srec[:], ssb[0:128:32, :])
                srec2 = normp.tile([16, 128], F32, tag="srec2")
                nc.vector.reciprocal_approx_fast(srec2[:], srec[:])
                srec2b = normp.tile([16, 128], BF16, tag="srec2b")
                nc.vector.tensor_copy(srec2b[:], srec2[:])
                # ONE scatter to a partition-0 row tile: rrow4[0, q, 128i+j]
                rrow4 = normp.tile([1, 4, 512], BF16, tag="rrow4")
                nc.scalar.dma_start(rrow4[:], srec2b[:])
                for q in range(4):
                    hp, hh = divmod(q, 2)
                    rb = normp.tile([64, 512], BF16, tag="rb")
                    nc.gpsimd.partition_broadcast(rb[:], rrow4[0:1, q, :])
                    src = (
                        opv[hp][64 * hh : 64 * hh + 64, :]
                        if last_chunk
                        else stg[q][:]
                    )
                    nc.vector.tensor_mul(
                        ats[qc][hp][64 * hh : 64 * hh + 64, :], src, rb[:]
                    )
                junk_fill(26 if last_chunk else 5)

            # ---- the hand-interleaved schedule ----
            qk0, v0 = phase1_groups(0)
            for g in qk0 + v0:
                g()
            p1_qk, p1_v = {}, {}
            for t in (1, 2, 3):
                p1_qk[t], p1_v[t] = phase1_groups(t)
            op0 = phase3_groups(0, nc.sync)
            op1 = phase3_groups(1, nc.sync)
            op2 = phase3_groups(2, nc.sync)
            op3 = phase3_groups(3, nc.sync)
            # boundary fills must come from the aux PSUM pool (V groups):
            # an out-proj boundary fill would hold an stp slot and stall the
            # next phase's first score tiles
            phase2(0, p1_qk[1], p1_v[1])
            phase2(1, p1_qk[2] + p1_v[2][:2] + op0, p1_v[2][2:])
            phase2(2, p1_qk[3] + op1, p1_v[3])
            phase2(3, op2, [], fill_delay=5)
            for g in op3:
                g()
    nc.compile()
    return nc


def _get_nc():
    if "nc" not in _CACHE:
        _CACHE["nc"] = _build()
    return _CACHE["nc"]


def _in_maps(x, Wq, Wk, Wv, Wo):
    bf = ml_dtypes.bfloat16
    x = np.asarray(x, dtype=np.float32)
    Wq = np.asarray(Wq, dtype=np.float32)
    Wk = np.asarray(Wk, dtype=np.float32)
    Wv = np.asarray(Wv, dtype=np.float32)
    Wo = np.asarray(Wo, dtype=np.float32)
    maps = []
    for core in range(N_CORES):
        b, g = divmod(core, 4)
        sl = slice(g * GC, (g + 1) * GC)
        # xs[p, tc, dt, t] = x[b, tc*512+t, dt*128+p]
        xsw = np.ascontiguousarray(
            x[b].reshape(4, 512, 8, 128).transpose(3, 0, 2, 1)
        ).astype(bf)
        # w[p, dt, c] = W[sl][c, dt*128+p]
        wqw = np.ascontiguousarray(
            Wq[sl].reshape(GC, 8, 128).transpose(2, 1, 0)
        ).astype(bf)
        wkw = np.ascontiguousarray(
            Wk[sl].reshape(GC, 8, 128).transpose(2, 1, 0)
        ).astype(bf)
        wvw = np.ascontiguousarray(
            Wv[sl].reshape(GC, 8, 128).transpose(2, 1, 0)
        ).astype(bf)
        # wo[p, ct, n] = Wo[n, g*256 + ct*128 + p]
        wow = np.ascontiguousarray(
            Wo[:, sl].reshape(D, 2, 128).transpose(2, 1, 0)
        ).astype(bf)
        maps.append(
            {"xs": xsw, "wqs": wqw, "wks": wkw, "wvs": wvw, "wos": wow}
        )
    return maps


def _run(x, Wq, Wk, Wv, Wo, **spmd_kwargs):
    nc = _get_nc()
    res = run_bass_kernel_spmd(
        nc, _in_maps(x, Wq, Wk, Wv, Wo), core_ids=list(range(N_CORES)), **spmd_kwargs
    )
    outs = [np.asarray(r["out"], dtype=np.float32) for r in res.results]
    full = np.stack(
        [
            outs[0] + outs[1] + outs[2] + outs[3],
            outs[4] + outs[5] + outs[6] + outs[7],
        ]
    )
    return full, res


def kernel(x, Wq, Wk, Wv, Wo):
    full, _ = _run(x, Wq, Wk, Wv, Wo)
    return full


# revision 47
# speedup vs baseline: 1.0003x; 1.0003x over previous
"""Causal multi-head self-attention (B=2, T=2048, D=1024, H=16) on 8 TRN2
NeuronCores.

Sharding (Megatron-style, hardcoded): core = 4*b + g where b in {0,1} is the
batch and g in {0..3} a group of 4 heads. Each core computes Q/K/V projections
for its head group from x[b], fused causal attention for those 4 heads, and a
partial output projection against its 256-column slice of Wo. The host sums
the 4 partial outputs per batch (the all-reduce after out_proj).

v4 design (all matmul operands bf16, fp32 PSUM accumulation):
 - Scores per (kti, head-pair): a 4-way tiled quad of M=64 matmuls covering
   all 16 PE subarray quadrants (rows split by head, cols by k-half); on
   causal-diagonal tiles both the matmul rhs (q columns) and output are
   sliced to the valid q range.
 - PV per (kti, head-pair): col-tiled M=64 pair (head0 -> PSUM rows 0:64,
   head1 -> 64:128) accumulating over kti; these co-issue as one PE stream.
 - Softmax denominators: 4-way col-tiled quad of M=1 ones-matmuls (one per
   head) accumulating into one PSUM bank at partitions {0,32,64,96}; the
   quad co-issues as a single stream.
 - The PE instruction stream is hand-interleaved: projection (next chunks)
   and output-projection (previous chunks) matmul groups are emitted INSIDE
   the attention kti loop, sized so the scalar engine's exp throughput (the
   attention pace-setter) is always covered by independent PE work:
     qc=0: Q/K projections of chunk 1; boundary: V of chunk 1
     qc=1: Q/K+V of chunk 2, then out-proj of chunk 0 (late slots + boundary)
     qc=2: Q/K of chunk 3 + out-proj of chunk 1; boundary: V of chunk 3
     qc=3: out-proj of chunk 2 (delayed past the normalize-2 window);
           out-proj of chunk 3 trails.
 - Normalize: evict PV+sums to SBUF, ONE strided-AP DMA gathers the four
   denominator rows {0,32,64,96} to [16,128], one reciprocal_approx_fast,
   cast to bf16, ONE DMA scatters to a [1,4,512] partition-0 row tile, then
   per head gpsimd partition_broadcast + DVE multiply (bf16 at). For the
   last chunk the multiply reads the PV accumulator directly from PSUM (no
   eviction). The normalize DMAs ride the scalar HWDGE queue; output writes
   ride sync - so the tiny hops never queue behind 256KB output writes.
 - The causal-triangle mask multiply runs on the DVE, NOT gpsimd: gpsimd
   then only ever executes partition_broadcast, avoiding the ~7us
   standard<->attn ucode library swap per normalize; a dummy broadcast at
   startup pre-loads the attn lib during the input-DMA dead time.
 - Keep-warm junk matmuls at phase boundaries and the tail park the PE
   through the normalize latency chain (an idle PE drops to a ~2x-slower
   DVFS p-state for ~3us after restart).
 - Output written in bf16 (host accumulates partials in fp32), halving the
   output DMA traffic.
 - Input DMAs split in 2-slice pieces and alternated across the two HWDGE
   queues in consumption order so the first projection matmul starts ~11us
   in (6.7us of that is fixed engine-preamble).
"""

import numpy as np
import ml_dtypes

import concourse.bass as bass
import concourse.tile as tile
from concourse import bacc, mybir
from concourse.bass_utils import run_bass_kernel_spmd

B, T, D, H, DH = 2, 2048, 1024, 16, 64
HPC = 4  # heads per core
GC = 256  # projection columns per core (HPC * DH)
N_CORES = 8
F32 = mybir.dt.float32
BF16 = mybir.dt.bfloat16
EXP = mybir.ActivationFunctionType.Exp

_CACHE = {}


def _build():
    nc = bacc.Bacc(
        "TRN2", target_bir_lowering=False, debug=False, num_devices=N_CORES
    )
    # Pre-swizzled bf16 inputs (host does transposes + cast):
    #   xs[p, tc, dt, t] = x[b, tc*512+t, dt*128+p]
    #   wq/wk/wv[p, dt, c] = W[g*256+c, dt*128+p]
    #   wo[p, ct, n] = Wo[n, g*256 + ct*128 + p]
    xs = nc.dram_tensor("xs", [128, 4, 8, 512], BF16, kind="ExternalInput").ap()
    wqs = nc.dram_tensor("wqs", [128, 8, GC], BF16, kind="ExternalInput").ap()
    wks = nc.dram_tensor("wks", [128, 8, GC], BF16, kind="ExternalInput").ap()
    wvs = nc.dram_tensor("wvs", [128, 8, GC], BF16, kind="ExternalInput").ap()
    wos = nc.dram_tensor("wos", [128, 2, D], BF16, kind="ExternalInput").ap()
    out = nc.dram_tensor("out", [T, D], BF16, kind="ExternalOutput").ap()

    with tile.TileContext(nc) as tc:
        with (
            tc.tile_pool(name="persist", bufs=1) as persist,
            tc.tile_pool(name="xtp", bufs=4) as xtp,
            tc.tile_pool(name="ptp", bufs=4) as ptp,
            tc.tile_pool(name="normp", bufs=2) as normp,
            tc.tile_pool(name="outp", bufs=2) as outp,
            # PSUM (8 banks): opv0+opv1+sums = 3, st-rotation (2 banks) x2 = 4
            # (shared by attention st tiles and phase-3 po3 tiles), aux = 1
            tc.tile_pool(name="pvp", bufs=1, space="PSUM") as pvp,
            tc.tile_pool(name="stp", bufs=2, space="PSUM") as stp,
            tc.tile_pool(name="auxp", bufs=1, space="PSUM") as auxp,
        ):
            wq = persist.tile([128, 8, GC], BF16, tag="wq")
            wk = persist.tile([128, 8, GC], BF16, tag="wk")
            wv = persist.tile([128, 8, GC], BF16, tag="wv")
            wo = persist.tile([128, 2, D], BF16, tag="wo")
            onesb = persist.tile([128, 4], BF16, tag="onesb")
            onesr = persist.tile([1, 64], BF16, tag="onesr")
            trimask = persist.tile([128, 2, 128], BF16, tag="trimask")
            # per-chunk projection outputs (separate tiles -> no false deps)
            qts = [
                persist.tile([128, 2, 512], BF16, tag=f"qt{t}", name=f"qt{t}")
                for t in range(4)
            ]
            kts = [
                persist.tile([128, 2, 512], BF16, tag=f"kt{t}", name=f"kt{t}")
                for t in range(4)
            ]
            vps = [
                persist.tile([128, 4, HPC, DH], BF16, tag=f"vp{t}", name=f"vp{t}")
                for t in range(4)
            ]
            # normalized attention output per (chunk, head-pair)
            ats = [
                [persist.tile([128, 512], BF16, tag=f"at{t}_{hp}",
                              name=f"at{t}_{hp}") for hp in range(2)]
                for t in range(4)
            ]
            # dedicated diagonal pt tiles per (offset j, head pair hp);
            # cols [0, 128j) are zeroed once and never rewritten
            ptdiag = [
                [persist.tile([128, 2, 512], BF16, tag=f"ptd{j}_{hp}",
                              name=f"ptd{j}_{hp}")
                 for hp in range(2)]
                for j in range(4)
            ]

            # Input DMAs: wq/wk/wv/wo on the scalar queue, x on sync, quartered
            # and interleaved so the first Q-proj matmul (needs wq[:,0,0:128] +
            # xt0[:,0,:]) starts a couple microseconds in.
            xt_all = []
            for t in range(4):
                xti = xtp.tile([128, 8, 512], BF16, tag="xt", name=f"xt{t}")
                xt_all.append(xti)
            # alternate the startup-critical pieces across BOTH HWDGE queues
            # in consumption order so neither queue gates the first matmuls
            nc.sync.dma_start(
                xt_all[0][:, 0:2, :], xs[:, 0, 0:2, :]
            )
            nc.scalar.dma_start(wq[:, 0:2, :], wqs[:, 0:2, :])
            nc.scalar.dma_start(wq[:, 2:4, :], wqs[:, 2:4, :])
            nc.sync.dma_start(
                xt_all[0][:, 2:4, :], xs[:, 0, 2:4, :]
            )
            nc.sync.dma_start(wq[:, 4:6, :], wqs[:, 4:6, :])
            nc.scalar.dma_start(
                xt_all[0][:, 4:6, :], xs[:, 0, 4:6, :]
            )
            nc.scalar.dma_start(wq[:, 6:8, :], wqs[:, 6:8, :])
            nc.sync.dma_start(
                xt_all[0][:, 6:8, :], xs[:, 0, 6:8, :]
            )
            nc.sync.dma_start(wk[:, 0:4, :], wks[:, 0:4, :])
            nc.scalar.dma_start(wk[:, 4:8, :], wks[:, 4:8, :])
            nc.sync.dma_start(xt_all[1][:], xs[:, 1])
            nc.scalar.dma_start(wv[:], wvs[:])
            nc.sync.dma_start(xt_all[2][:], xs[:, 2])
            nc.sync.dma_start(xt_all[3][:], xs[:, 3])
            nc.scalar.dma_start(wo[:], wos[:])

            nc.vector.memset(onesb[:], 1.0)
            nc.vector.memset(onesr[:], 1.0)
            # trimask[r, hh, y] = 1 if y >= r else 0
            nc.vector.memset(trimask[:], 1.0)
            nc.gpsimd.affine_select(
                out=trimask[:],
                in_=trimask[:],
                compare_op=mybir.AluOpType.is_ge,
                fill=0.0,
                base=0,
                pattern=[[0, 2], [1, 128]],
                channel_multiplier=-1,
            )
            for j in range(1, 4):
                for hp in range(2):
                    nc.vector.memset(ptdiag[j][hp][:, :, 0 : 128 * j], 0.0)

            # dummy broadcast: pulls the gpsimd attn ucode lib (the one
            # holding InstPartitionBroadcast) into iram during the startup
            # DMA dead time, so normalize(0) doesn't eat the ~7us lib load
            dumb = persist.tile([64, 4], BF16, tag="dumb")
            nc.gpsimd.partition_broadcast(dumb[:], onesb[0:1, :])

            # PE warm-up: as soon as the first x slice lands, spin throwaway
            # matmuls so the DVFS ramp is fully up when the projections start
            # (a cold PE streams at ~half rate for its first ~3us)
            jw = pvp.tile([128, 512], F32, tag="sums", name="junkw")
            for i in range(10):
                nc.tensor.matmul(
                    jw[:],
                    xt_all[0][:, 0, 0:128],
                    xt_all[0][:, 1, :],
                    start=(i == 0),
                    stop=(i == 9),
                )

            # ---- phase 1 groups: projections for x chunk tci ----
            def phase1_groups(tci):
                xt = xt_all[tci]

                def qk_group(w_sb, dst, ct):
                    def emit():
                        ps = auxp.tile([128, 512], F32, tag="aux", name="ps_qk")
                        for di in range(8):
                            nc.tensor.matmul(
                                ps[:],
                                w_sb[:, di, ct * 128 : (ct + 1) * 128],
                                xt[:, di, :],
                                start=(di == 0),
                                stop=(di == 7),
                            )
                        nc.vector.tensor_copy(dst[:, ct, :], ps[:])
                    return emit

                def v_group(tt):
                    def emit():
                        ps = auxp.tile([128, 256], F32, tag="aux", name="ps_v")
                        for di in range(8):
                            nc.tensor.matmul(
                                ps[:],
                                xt[:, di, tt * 128 : (tt + 1) * 128],
                                wv[:, di, :],
                                start=(di == 0),
                                stop=(di == 7),
                            )
                        nc.vector.tensor_copy(
                            vps[tci][:, tt, :, :],
                            ps[:].rearrange("p (h d) -> p h d", h=HPC),
                        )
                    return emit

                qk = [
                    qk_group(w_sb, dst, ct)
                    for w_sb, dst in ((wq, qts[tci]), (wk, kts[tci]))
                    for ct in range(2)
                ]
                vg = [v_group(tt) for tt in range(4)]
                return qk, vg

            # ---- phase 3 groups: output projection for q chunk qc ----
            def phase3_groups(qc, dma_eng):
                def o_group(tt):
                    def emit():
                        qti = qc * 4 + tt
                        po3 = stp.tile([128, 2, 512], F32, tag="st", name="po3")
                        # ctt-outer: the two nn matmuls of one ctt share the
                        # same stationary lhs -> back-to-back weight reuse
                        for ctt in range(2):
                            for nn in range(2):
                                nc.tensor.matmul(
                                    po3[:, nn, :],
                                    ats[qc][ctt][:, tt * 128 : (tt + 1) * 128],
                                    wo[:, ctt, nn * 512 : (nn + 1) * 512],
                                    start=(ctt == 0),
                                    stop=(ctt == 1),
                                )
                        ot = outp.tile([128, 2, 512], BF16, tag="ot")
                        nc.vector.tensor_copy(ot[:], po3[:])
                        dma_eng.dma_start(
                            out[qti * 128 : (qti + 1) * 128, :].rearrange(
                                "q (a n) -> q a n", a=2
                            ),
                            ot[:],
                        )
                    return emit

                return [o_group(tt) for tt in range(4)]

            # keep-warm filler: the PE drops to a ~2x-slower DVFS p-state
            # after idling, so park it on throwaway matmuls (into the
            # just-freed sums bank) across the normalize latency chain
            # instead of letting it stall cold
            def junk_fill(n, pool=None):
                # boundary fills target the sums bank (waits the ssb
                # eviction ~1us; an stp-tagged target was measured worse -
                # it steals a rotation slot from the next phase's score
                # tiles). The tail passes the aux pool instead: free and
                # dependency-clear in the last phase, so the filler starts
                # the instant the attention matmuls end.
                jt = (pool or pvp).tile(
                    [128, 512], F32,
                    tag="aux" if pool is not None else "sums",
                    name="junk",
                )
                for i in range(n):
                    nc.tensor.matmul(
                        jt[:],
                        wo[:, 0, 0:128],
                        wo[:, 1, 0:512],
                        start=(i == 0),
                        stop=(i == n - 1),
                    )

            # ---- phase 2: attention for q chunk qc, interleaving fills ----
            def phase2(qc, fills, boundary, fill_delay=0):
                q0 = qc * 512
                n_kt = 4 * (qc + 1)
                opv = [
                    pvp.tile([128, 512], F32, tag=f"opv{hp}", name=f"opv{hp}")
                    for hp in range(2)
                ]
                sums = pvp.tile([128, 512], F32, tag="sums", name="sums")

                def scores_exp(kti):
                    diag = kti >= 4 * qc
                    j = kti - 4 * qc if diag else 0
                    off = 128 * j
                    tci, tk = divmod(kti, 4)
                    k0 = tk * 128
                    pts = []
                    for hp in range(2):
                        st = stp.tile([128, 2, 512], F32, tag="st", name="st")
                        for hh in range(2):
                            po = 64 * hh
                            # one 64x128 weight tile (row-half po, full cols)
                            # covers both k-halves of this head's score tile
                            nc.tensor.matmul(
                                st[:, hh, off:],
                                kts[tci][po : po + 64, hp, k0 : k0 + 128],
                                qts[qc][po : po + 64, hp, off:],
                                start=True,
                                stop=True,
                                tile_position=(po, 0),
                            )
                        if diag:
                            pt = ptdiag[j][hp]
                            nc.scalar.activation(
                                pt[:, :, off:], st[:, :, off:], EXP, scale=0.125
                            )
                            # mask on DVE, NOT gpsimd: keeps the Pool engine
                            # exclusively on the attn ucode lib (a
                            # standard<->attn lib swap costs ~7us per load)
                            nc.vector.tensor_mul(
                                pt[:, :, off : off + 128],
                                pt[:, :, off : off + 128],
                                trimask[:],
                            )
                        else:
                            pt = ptp.tile([128, 2, 512], BF16, tag="pt")
                            nc.scalar.activation(pt[:], st[:], EXP, scale=0.125)
                        pts.append(pt)
                    return pts

                def pv_sums(kti, pts, last):
                    first = kti == 0
                    tci, tk = divmod(kti, 4)
                    for hp in range(2):
                        for hh in range(2):
                            nc.tensor.matmul(
                                opv[hp][64 * hh : 64 * hh + 64, :],
                                vps[tci][:, tk, 2 * hp + hh, :],
                                pts[hp][:, hh, :],
                                start=first,
                                stop=last,
                                tile_position=(0, 64 * hh),
                            )
                    for q in range(4):
                        hp, hh = divmod(q, 2)
                        nc.tensor.matmul(
                            sums[32 * q : 32 * q + 1, :],
                            onesb[:, q : q + 1],
                            pts[hp][:, hh, :],
                            start=first,
                            stop=last,
                            tile_position=(0, 32 * q),
                        )

                def want(halfstep):
                    lo = 2 * fill_delay
                    if halfstep <= lo:
                        return 0
                    return (halfstep - lo) * len(fills) // (2 * n_kt - lo)

                emitted = 0
                prev = None
                for kti in range(n_kt):
                    cur = scores_exp(kti)
                    while emitted < want(2 * kti + 1):
                        fills[emitted]()
                        emitted += 1
                    if prev is not None:
                        pv_sums(kti - 1, prev, last=False)
                    prev = cur
                    while emitted < want(2 * kti + 2):
                        fills[emitted]()
                        emitted += 1
                pv_sums(n_kt - 1, prev, last=True)
                while emitted < len(fills):
                    fills[emitted]()
                    emitted += 1

                # evict sums FIRST (it heads the normalize critical chain),
                # then the PV accumulators so the next chunk's PV can start
                # during normalization. Last chunk: no next chunk -> the
                # multiplies read the PV accumulator straight from PSUM.
                last_chunk = qc == 3
                stg = []
                if not last_chunk:
                    for q in range(4):
                        hp, hh = divmod(q, 2)
                        s = normp.tile([64, 512], F32, tag=f"stg{q}",
                                       name=f"stg{q}")
                        nc.vector.tensor_copy(
                            s[:], opv[hp][64 * hh : 64 * hh + 64, :]
                        )
                        stg.append(s)
                ssb = normp.tile([128, 512], F32, tag="ssb")
                nc.vector.tensor_copy(ssb[:], sums[:])

                # boundary fills: PE work covering the normalize latency chain
                for g in boundary:
                    g()

                # ONE strided gather of denominator rows {0,32,64,96} ->
                # srec[4q+i, j] = ssb[32q, 128i+j]
                # (on the scalar HWDGE queue: the sync queue carries the
                # bulk output writes, which would delay these tiny hops)
                srec = normp.tile([16, 128], F32, tag="srec")
                nc.scalar.dma_start(srec[:], ssb[0:128:32, :])
                srec2 = normp.tile([16, 128], F32, tag="srec2")
                nc.vector.reciprocal_approx_fast(srec2[:], srec[:])
                srec2b = normp.tile([16, 128], BF16, tag="srec2b")
                nc.vector.tensor_copy(srec2b[:], srec2[:])
                # ONE scatter to a partition-0 row tile: rrow4[0, q, 128i+j]
                rrow4 = normp.tile([1, 4, 512], BF16, tag="rrow4")
                nc.scalar.dma_start(rrow4[:], srec2b[:])
                for q in range(4):
                    hp, hh = divmod(q, 2)
                    rb = normp.tile([64, 512], BF16, tag="rb")
                    nc.gpsimd.partition_broadcast(rb[:], rrow4[0:1, q, :])
                    src = (
                        opv[hp][64 * hh : 64 * hh + 64, :]
                        if last_chunk
                        else stg[q][:]
                    )
                    nc.vector.tensor_mul(
                        ats[qc][hp][64 * hh : 64 * hh + 64, :], src, rb[:]
                    )
                if last_chunk:
                    junk_fill(26, pool=auxp)
                else:
                    junk_fill(5)

            # ---- the hand-interleaved schedule ----
            qk0, v0 = phase1_groups(0)
            for g in qk0 + v0:
                g()
            p1_qk, p1_v = {}, {}
            for t in (1, 2, 3):
                p1_qk[t], p1_v[t] = phase1_groups(t)
            op0 = phase3_groups(0, nc.sync)
            op1 = phase3_groups(1, nc.sync)
            op2 = phase3_groups(2, nc.sync)
            op3 = phase3_groups(3, nc.sync)
            # boundary fills must come from the aux PSUM pool (V groups):
            # an out-proj boundary fill would hold an stp slot and stall the
            # next phase's first score tiles
            phase2(0, p1_qk[1], p1_v[1])
            phase2(1, p1_qk[2] + p1_v[2][:2] + op0, p1_v[2][2:])
            phase2(2, p1_qk[3] + op1, p1_v[3])
            phase2(3, op2, [], fill_delay=5)
            for g in op3:
                g()
    nc.compile()
    return nc


def _get_nc():
    if "nc" not in _CACHE:
        _CACHE["nc"] = _build()
    return _CACHE["nc"]


def _in_maps(x, Wq, Wk, Wv, Wo):
    bf = ml_dtypes.bfloat16
    x = np.asarray(x, dtype=np.float32)
    Wq = np.asarray(Wq, dtype=np.float32)
    Wk = np.asarray(Wk, dtype=np.float32)
    Wv = np.asarray(Wv, dtype=np.float32)
    Wo = np.asarray(Wo, dtype=np.float32)
    maps = []
    for core in range(N_CORES):
        b, g = divmod(core, 4)
        sl = slice(g * GC, (g + 1) * GC)
        # xs[p, tc, dt, t] = x[b, tc*512+t, dt*128+p]
        xsw = np.ascontiguousarray(
            x[b].reshape(4, 512, 8, 128).transpose(3, 0, 2, 1)
        ).astype(bf)
        # w[p, dt, c] = W[sl][c, dt*128+p]
        wqw = np.ascontiguousarray(
            Wq[sl].reshape(GC, 8, 128).transpose(2, 1, 0)
        ).astype(bf)
        wkw = np.ascontiguousarray(
            Wk[sl].reshape(GC, 8, 128).transpose(2, 1, 0)
        ).astype(bf)
        wvw = np.ascontiguousarray(
            Wv[sl].reshape(GC, 8, 128).transpose(2, 1, 0)
        ).astype(bf)
        # wo[p, ct, n] = Wo[n, g*256 + ct*128 + p]
        wow = np.ascontiguousarray(
            Wo[:, sl].reshape(D, 2, 128).transpose(2, 1, 0)
        ).astype(bf)
        maps.append(
            {"xs": xsw, "wqs": wqw, "wks": wkw, "wvs": wvw, "wos": wow}
        )
    return maps


def _run(x, Wq, Wk, Wv, Wo, **spmd_kwargs):
    nc = _get_nc()
    res = run_bass_kernel_spmd(
        nc, _in_maps(x, Wq, Wk, Wv, Wo), core_ids=list(range(N_CORES)), **spmd_kwargs
    )
    outs = [np.asarray(r["out"], dtype=np.float32) for r in res.results]
    full = np.stack(
        [
            outs[0] + outs[1] + outs[2] + outs[3],
            outs[4] + outs[5] + outs[6] + outs[7],
        ]
    )
    return full, res


def kernel(x, Wq, Wk, Wv, Wo):
    full, _ = _run(x, Wq, Wk, Wv, Wo)
    return full
